# revision 1
# baseline (speedup 1.0000x reference)
"""Trainium2 Bass kernel for nn_Decoder_17076789969159 (gnn_message_passing).

Sharding: data-parallel over batch (2 groups of 4 cores); within a group the
permuted point axis of each space-filling-curve order is split in 4 contiguous
chunks. Per-order conv outputs are AllGather'd inside the group; the next
layer gathers its inputs with host-composed inverse-permutation indices, so
no scatter is ever needed on device.

Row gathers use the walrus DGE indirect DMA (one [128,1] offset column per
call, point-major result) followed by PE transposes to channel-major.

Self-contained: hardcodes all shapes from the problem spec.
"""

import os
import numpy as np
import ml_dtypes

BF16 = ml_dtypes.bfloat16

# Problem shapes (hardcoded per contract)
B, N, LL, O, KK, PAD = 2, 32768, 8192, 3, 9, 4
C = 256          # conv output channels
C1R = 304        # conv1 input channels (256 xi + 48 low)
CH1 = 384        # padded h row (3 * 128)
CLS = 13
NCORES, GRP = 8, 4
QN = N // GRP            # 8192 permuted positions per core per order
NPT = 512                # points per conv tile
NT = QN // NPT           # 16 conv tiles per (order) per core
NTS0 = N // NPT          # 64 stage0 tiles (full batch, replicated in group)
NTF = QN // NPT          # 16 final tiles (core's original-index quarter)
NG = NPT + 2 * PAD       # 520 needed gather cols
KG = 5                   # gather columns per conv tile (5*128 = 640 rows)
NGI = KG * 128
EPS = 1e-5

_CACHE = {}


# ----------------------------------------------------------------------------
# host-side preparation
# ----------------------------------------------------------------------------

def _cols(vals):
    """index vector of length k*128 -> [128, k] int32 column layout."""
    v = np.asarray(vals, np.int32)
    return np.ascontiguousarray(v.reshape(-1, 128).T)


def _bn_affine(g, b, m, v):
    s = g / np.sqrt(v + EPS)
    return s.astype(np.float32), (b - m * s).astype(np.float32)


def _prep_shared(inp):
    sh = {}
    w1 = np.asarray(inp["w1_w"], np.float32)   # [256, 304, 9]
    w1p = np.zeros((128, KK * 3 * C), np.float32)
    for j in range(KK):
        for kc in range(3):
            ci0 = kc * 128
            ncid = min(128, C1R - ci0)
            if ncid > 0:
                blk = w1[:, ci0:ci0 + ncid, j].T  # [ncid, 256]
                w1p[:ncid, (j * 3 + kc) * C:(j * 3 + kc) * C + C] = blk
    sh["w1p"] = w1p.astype(BF16)

    w2 = np.asarray(inp["w2_w"], np.float32)   # [256, 256, 9]
    w2p = np.zeros((128, KK * 2 * C), np.float32)
    for j in range(KK):
        for kc in range(2):
            blk = w2[:, kc * 128:(kc + 1) * 128, j].T
            w2p[:, (j * 2 + kc) * C:(j * 2 + kc) * C + C] = blk
    sh["w2p"] = w2p.astype(BF16)

    sh["c1"] = np.asarray(inp["conv1_w"], np.float32).T.astype(BF16)  # [128,48]

    ow = np.asarray(inp["out_w"], np.float32)  # [13, 256]
    owp = np.zeros((128, 2 * CLS), np.float32)
    for g in range(2):
        owp[:, g * CLS:(g + 1) * CLS] = ow[:, g * 128:(g + 1) * 128].T
    sh["outw"] = owp.astype(BF16)

    sh["eye"] = np.eye(128, dtype=BF16)
    sh["eyef"] = np.eye(128, dtype=np.float32)

    bnv = np.zeros((128, 11), np.float32)
    s1, b1 = _bn_affine(inp["bn1_g"], inp["bn1_b"], inp["bn1_m"], inp["bn1_v"])
    bnv[:48, 0], bnv[:48, 1] = s1, b1
    sc, bc = _bn_affine(inp["bnc1_g"], inp["bnc1_b"], inp["bnc1_m"], inp["bnc1_v"])
    bc = bc + np.asarray(inp["w1_b"], np.float32) * sc
    for g in range(2):
        bnv[:, 2 + g] = sc[g * 128:(g + 1) * 128] / 3.0
        bnv[:, 4 + g] = bc[g * 128:(g + 1) * 128]
    sc2, bc2 = _bn_affine(inp["bnc2_g"], inp["bnc2_b"], inp["bnc2_m"], inp["bnc2_v"])
    bc2 = bc2 + np.asarray(inp["w2_b"], np.float32) * sc2
    for g in range(2):
        bnv[:, 6 + g] = sc2[g * 128:(g + 1) * 128] / 3.0
        bnv[:, 8 + g] = bc2[g * 128:(g + 1) * 128]
    bnv[:CLS, 10] = np.asarray(inp["out_b"], np.float32)
    sh["bnvec"] = bnv

    # interp tables: per chunk 8 cols (4 of i0, 4 of i1)
    pos = np.arange(N, dtype=np.float64) * ((LL - 1) / (N - 1))
    i0 = np.floor(pos).astype(np.int64)
    i1 = np.minimum(i0 + 1, LL - 1)
    t = (pos - i0).astype(np.float32)
    icols = []
    for ch in range(NTS0):
        s = slice(ch * NPT, (ch + 1) * NPT)
        icols.append(_cols(np.concatenate([i0[s], i1[s]])))
    sh["iidx"] = np.concatenate(icols, axis=1)           # [128, NTS0*8]
    tt = np.zeros((128, NTS0 * 4), np.float32)
    for ch in range(NTS0):
        for s in range(4):
            tt[:, ch * 4 + s] = t[ch * NPT + s * 128: ch * NPT + (s + 1) * 128]
    sh["tt"] = tt
    return sh


def _prep_core(inp, c):
    b, q = c // GRP, c % GRP
    pc = {}
    x = np.asarray(inp["x"], np.float32)
    pc["xt"] = np.ascontiguousarray(x[b].T).astype(BF16)          # [8192, 256]
    pc["llf"] = np.asarray(inp["low_level_feat"], np.float32)[b].astype(BF16)

    rot = np.asarray(inp["rotations"], np.int64)[:, b, :]          # [O, N]
    inv = np.stack([np.argsort(rot[o], kind="stable") for o in range(O)])

    # geometry weights in permuted space, OOB taps zeroed
    coords = np.asarray(inp["coords"], np.float32)[b]              # [3, N]
    dist = np.asarray(inp["distances"], np.float32)[b]             # [O, N]
    wall = np.zeros((O, KK, N), np.float32)
    ar = np.arange(N)
    for o in range(O):
        co = coords[:, rot[o]]                                     # [3, N]
        d = dist[o]
        dp = np.pad(d, (PAD, PAD))
        cp = np.pad(co, ((0, 0), (PAD, PAD)))
        for j in range(KK):
            dd = (dp[j:j + N] - d) ** 2
            dc = ((cp[:, j:j + N] - co) ** 2).sum(0)
            w = np.exp(-(dd + dc))
            pin = ar + j - PAD
            w[(pin < 0) | (pin >= N)] = 0.0
            wall[o, j] = w
    # per-core chunk-blocked, pre-broadcast to 128 partitions
    wgt = np.zeros((O * NT, KK * NPT), np.float32)
    for o in range(O):
        for tch in range(NT):
            base = q * QN + tch * NPT
            wgt[o * NT + tch] = wall[o, :, base:base + NPT].reshape(KK * NPT)
    pc["wgt"] = np.ascontiguousarray(
        np.broadcast_to(wgt.astype(BF16)[:, None, :], (O * NT, 128, KK * NPT)))

    # L1/L2 gather indices (same h-row values: rows are original points)
    gcols, ccols = [], []
    mar = np.arange(NGI)
    for o in range(O):
        for tch in range(NT):
            base = q * QN + tch * NPT
            pp = base - PAD + mar
            valid = (pp >= 0) & (pp < N) & (mar < NG)
            ppc = np.clip(pp, 0, N - 1)
            gcols.append(_cols(np.where(valid, rot[o][ppc], 0)))
            for os_ in range(O):
                ccols.append(_cols(np.where(valid, inv[os_][rot[o][ppc]], 0)))
    pc["gidx"] = np.concatenate(gcols, axis=1)    # [128, O*NT*KG]
    pc["cidx"] = np.concatenate(ccols, axis=1)    # [128, O*NT*O*KG]

    fcols = []
    for tch in range(NTF):
        base = q * QN + tch * NPT
        for os_ in range(O):
            fcols.append(_cols(inv[os_][base:base + NPT]))
    pc["fidx"] = np.concatenate(fcols, axis=1)    # [128, NTF*O*4]
    return pc


# ----------------------------------------------------------------------------
# device program
# ----------------------------------------------------------------------------

def _build_nc():
    import concourse.bacc as bacc
    import concourse.bass as bass
    import concourse.tile as tile
    import concourse.mybir as mybir

    dt = mybir.dt
    AF = mybir.ActivationFunctionType
    IOA = bass.IndirectOffsetOnAxis
    nocc = os.environ.get("KNOCC", "0") == "1"
    nc = bacc.Bacc("TRN2", target_bir_lowering=False, debug=False,
                   num_devices=1 if nocc else NCORES)

    def EIN(name, shape, dty):
        return nc.dram_tensor(name, list(shape), dty, kind="ExternalInput")

    xt = EIN("xt", [LL, C], dt.bfloat16)
    llf = EIN("llf", [128, N], dt.bfloat16)
    w1p = EIN("w1p", [128, KK * 3 * C], dt.bfloat16)
    w2p = EIN("w2p", [128, KK * 2 * C], dt.bfloat16)
    c1 = EIN("c1", [128, 48], dt.bfloat16)
    outw = EIN("outw", [128, 2 * CLS], dt.bfloat16)
    eye = EIN("eye", [128, 128], dt.bfloat16)
    eyef = EIN("eyef", [128, 128], dt.float32)
    bnvec = EIN("bnvec", [128, 11], dt.float32)
    wgt = EIN("wgt", [O * NT, 128, KK * NPT], dt.bfloat16)
    gidx = EIN("gidx", [128, O * NT * KG], dt.int32)
    cidx = EIN("cidx", [128, O * NT * O * KG], dt.int32)
    fidx = EIN("fidx", [128, NTF * O * 4], dt.int32)
    iidx = EIN("iidx", [128, NTS0 * 8], dt.int32)
    tt = EIN("tt", [128, NTS0 * 4], dt.float32)

    out = nc.dram_tensor("out", [CLS, QN], dt.float32, kind="ExternalOutput")
    dbg = os.environ.get("KDBG", "0") == "1"
    if dbg:
        dbg_h = nc.dram_tensor("dbg_h", [N, CH1], dt.bfloat16, kind="ExternalOutput")
        dbg_y1in0 = nc.dram_tensor("dbg_y1in0", [QN, C], dt.bfloat16, kind="ExternalOutput")
        dbg_y1all0 = nc.dram_tensor("dbg_y1all0", [N, C], dt.bfloat16, kind="ExternalOutput")
        dbg_y2in0 = nc.dram_tensor("dbg_y2in0", [QN, C], dt.bfloat16, kind="ExternalOutput")

    RG = [[0, 1, 2, 3], [4, 5, 6, 7]]

    with tile.TileContext(nc) as tc:
        with (
            tc.tile_pool(name="dram", bufs=1, space="DRAM") as dpool,
            tc.tile_pool(name="res", bufs=1) as res,
            tc.tile_pool(name="wk", bufs=3) as wk,
            tc.tile_pool(name="wc", bufs=2) as wc,
            tc.tile_pool(name="ps", bufs=4, space="PSUM") as psp,
            tc.tile_pool(name="pt", bufs=4, space="PSUM") as ptp,
        ):
            h_t = dpool.tile([N, CH1], dt.bfloat16, tag="h")
            y1in = [dpool.tile([QN, C], dt.bfloat16, tag=f"y1in{o}",
                               name=f"y1in{o}") for o in range(O)]
            y1all = [dpool.tile([N, C], dt.bfloat16, tag=f"y1all{o}",
                                name=f"y1all{o}") for o in range(O)]
            y2in = [dpool.tile([QN, C], dt.bfloat16, tag=f"y2in{o}",
                               name=f"y2in{o}") for o in range(O)]
            y2all = [dpool.tile([N, C], dt.bfloat16, tag=f"y2all{o}",
                                name=f"y2all{o}") for o in range(O)]

            # resident SBUF constants
            def LOAD(src, shape, dty, tag):
                tl = res.tile(shape, dty, tag=tag, name=tag)
                nc.sync.dma_start(tl[:], src[:])
                return tl
            w1s = LOAD(w1p, [128, KK * 3 * C], dt.bfloat16, "w1s")
            w2s = LOAD(w2p, [128, KK * 2 * C], dt.bfloat16, "w2s")
            c1s = LOAD(c1, [128, 48], dt.bfloat16, "c1s")
            ows = LOAD(outw, [128, 2 * CLS], dt.bfloat16, "ows")
            eys = LOAD(eye, [128, 128], dt.bfloat16, "eys")
            eysf = LOAD(eyef, [128, 128], dt.float32, "eysf")
            bns = LOAD(bnvec, [128, 11], dt.float32, "bns")
            gis = LOAD(gidx, [128, O * NT * KG], dt.int32, "gis")
            cis = LOAD(cidx, [128, O * NT * O * KG], dt.int32, "cis")
            fis = LOAD(fidx, [128, NTF * O * 4], dt.int32, "fis")
            iis = LOAD(iidx, [128, NTS0 * 8], dt.int32, "iis")
            tts = LOAD(tt, [128, NTS0 * 4], dt.float32, "tts")

            def rows_pm(dram_tile, base, nrows, rowlen):
                """point-major SBUF tile [128, nrows//128, rowlen] <-> dram rows."""
                return bass.AP(dram_tile.tensor, base * rowlen,
                               [[rowlen, 128], [128 * rowlen, nrows // 128],
                                [1, rowlen]])

            def igather(dst2d, src, idx_col):
                nc.gpsimd.indirect_dma_start(
                    out=dst2d, out_offset=None, in_=src[:, :],
                    in_offset=IOA(ap=idx_col, axis=0))

            # ---------------- stage 0: build h ----------------
            for ch in range(NTS0):
                xg = wk.tile([128, 8, C], dt.bfloat16, tag="xg")
                for k in range(8):
                    igather(xg[:, k, :], xt, iis[:, ch * 8 + k:ch * 8 + k + 1])
                hrow = wk.tile([128, 4, CH1], dt.bfloat16, tag="hrow")
                xd = wk.tile([128, 4, C], dt.float32, tag="xd")
                nc.vector.tensor_sub(xd[:], xg[:, 4:8, :], xg[:, 0:4, :])
                for s in range(4):
                    nc.vector.tensor_scalar_mul(
                        xd[:, s, :], xd[:, s, :], tts[:, ch * 4 + s:ch * 4 + s + 1])
                nc.vector.tensor_add(hrow[:, :, 0:C], xg[:, 0:4, :], xd[:])

                lsb = wk.tile([128, NPT], dt.bfloat16, tag="lsb")
                nc.sync.dma_start(lsb[:], llf[:, ch * NPT:(ch + 1) * NPT])
                p48 = psp.tile([48, NPT], dt.float32, tag="pc")
                nc.tensor.matmul(p48[:], c1s[:], lsb[:], start=True, stop=True)
                low = wk.tile([48, NPT], dt.bfloat16, tag="low")
                nc.scalar.activation(low[:], p48[:], AF.Relu,
                                     bias=bns[:48, 1:2], scale=bns[:48, 0:1])
                for s in range(4):
                    ptt = ptp.tile([128, 48], dt.bfloat16, tag="pt")
                    nc.tensor.transpose(ptt[:], low[:48, s * 128:(s + 1) * 128],
                                        eys[:48, :48])
                    nc.vector.tensor_copy(hrow[:, s, C:C + 48], ptt[:])
                nc.vector.memset(hrow[:, :, C + 48:CH1], 0)
                nc.sync.dma_start(rows_pm(h_t, ch * NPT, NPT, CH1), hrow[:])

            # ---------------- conv layer helper ----------------
            def conv_layer(yin, wsb_pack, nkc, gather_one, after_order=None):
                for o in range(O):
                    for tch in range(NT):
                        blk = o * NT + tch
                        hx = gather_one(o, tch)
                        wsb = wc.tile([128, KK * NPT], dt.bfloat16, tag="wsb")
                        nc.sync.dma_start(wsb[:], wgt[blk, :, :])
                        pg = [psp.tile([128, NPT], dt.float32, tag="pc",
                                       name=f"pg{g}") for g in range(2)]
                        for j in range(KK):
                            xw = wk.tile([128, nkc, NPT], dt.bfloat16, tag="xw")
                            for kc in range(nkc):
                                nc.vector.tensor_mul(
                                    xw[:, kc, :], hx[:, kc, j:j + NPT],
                                    wsb[:, j * NPT:(j + 1) * NPT])
                            for g in range(2):
                                for kc in range(nkc):
                                    wsl = wsb_pack[:, ((j * nkc + kc) * C + g * 128):
                                                   ((j * nkc + kc) * C + g * 128 + 128)]
                                    nc.tensor.matmul(
                                        pg[g][:], wsl, xw[:, kc, :],
                                        start=(j == 0 and kc == 0),
                                        stop=(j == KK - 1 and kc == nkc - 1))
                        ysb = wk.tile([128, 2, NPT], dt.bfloat16, tag="ysb")
                        for g in range(2):
                            nc.scalar.activation(ysb[:, g, :], pg[g][:], AF.Copy)
                        yT = wk.tile([128, 4, C], dt.bfloat16, tag="yT")
                        for g in range(2):
                            for s in range(4):
                                ptt = ptp.tile([128, 128], dt.bfloat16, tag="pt")
                                nc.tensor.transpose(
                                    ptt[:], ysb[:, g, s * 128:(s + 1) * 128], eys[:])
                                nc.vector.tensor_copy(
                                    yT[:, s, g * 128:(g + 1) * 128], ptt[:])
                        nc.sync.dma_start(rows_pm(yin[o], tch * NPT, NPT, C), yT[:])
                    if after_order is not None:
                        after_order(o)

            # L1: gather padded-384 rows of h, transpose to channel-major
            def gather_l1(o, tch):
                blk = o * NT + tch
                g1pm = wk.tile([128, KG, CH1], dt.bfloat16, tag="g1pm")
                for k in range(KG):
                    igather(g1pm[:, k, :], h_t, gis[:, blk * KG + k:blk * KG + k + 1])
                g1 = wk.tile([128, 3, NGI], dt.bfloat16, tag="g1")
                for kc in range(3):
                    for k in range(KG):
                        ptt = ptp.tile([128, 128], dt.bfloat16, tag="pt")
                        nc.tensor.transpose(
                            ptt[:], g1pm[:, k, kc * 128:(kc + 1) * 128], eys[:])
                        nc.vector.tensor_copy(
                            g1[:, kc, k * 128:(k + 1) * 128], ptt[:])
                return g1

            def ag1(o):
                if nocc:
                    return
                nc.gpsimd.collective_compute(
                    "AllGather", mybir.AluOpType.bypass, replica_groups=RG,
                    ins=[y1in[o].opt()], outs=[y1all[o].opt()])

            conv_layer(y1in, w1s, 3, gather_l1, after_order=ag1)

            # L2: composed gathers from 3 slabs, sum pm, transpose + bn1+relu
            def gather_l2(o, tch):
                blk = (o * NT + tch) * O
                gs = []
                for os_ in range(O):
                    g2pm = wk.tile([128, KG, C], dt.bfloat16, tag="g2pm",
                                   name=f"g2pm{os_}")
                    for k in range(KG):
                        igather(g2pm[:, k, :], y1all[os_],
                                cis[:, (blk + os_) * KG + k:(blk + os_) * KG + k + 1])
                    gs.append(g2pm)
                s12 = wk.tile([128, KG, C], dt.float32, tag="s12")
                nc.vector.tensor_add(s12[:], gs[0][:], gs[1][:])
                nc.vector.tensor_add(s12[:], s12[:], gs[2][:])
                hx = wk.tile([128, 2, NGI], dt.bfloat16, tag="hx")
                for g in range(2):
                    for k in range(KG):
                        ptt = ptp.tile([128, 128], dt.float32, tag="pt")
                        nc.tensor.transpose(
                            ptt[:], s12[:, k, g * 128:(g + 1) * 128], eysf[:])
                        nc.scalar.activation(hx[:, g, k * 128:(k + 1) * 128],
                                             ptt[:], AF.Relu,
                                             bias=bns[:, 4 + g:5 + g],
                                             scale=bns[:, 2 + g:3 + g])
                return hx

            def ag2(o):
                if nocc:
                    return
                nc.gpsimd.collective_compute(
                    "AllGather", mybir.AluOpType.bypass, replica_groups=RG,
                    ins=[y2in[o].opt()], outs=[y2all[o].opt()])

            conv_layer(y2in, w2s, 2, gather_l2, after_order=ag2)

            if dbg:
                nc.sync.dma_start(dbg_h[:, :], h_t[:, :])
                nc.sync.dma_start(dbg_y1in0[:, :], y1in[0][:, :])
                nc.sync.dma_start(dbg_y1all0[:, :], y1all[0][:, :])
                nc.sync.dma_start(dbg_y2in0[:, :], y2in[0][:, :])

            # ---------------- final: bn2+relu+proj ----------------
            for tch in range(NTF):
                gs = []
                for os_ in range(O):
                    blk = tch * O + os_
                    g3pm = wk.tile([128, 4, C], dt.bfloat16, tag="g3pm",
                                   name=f"g3pm{os_}")
                    for k in range(4):
                        igather(g3pm[:, k, :], y2all[os_],
                                fis[:, blk * 4 + k:blk * 4 + k + 1])
                    gs.append(g3pm)
                s3 = wk.tile([128, 4, C], dt.float32, tag="s3")
                nc.vector.tensor_add(s3[:], gs[0][:], gs[1][:])
                nc.vector.tensor_add(s3[:], s3[:], gs[2][:])
                h2 = wk.tile([128, 2, NPT], dt.bfloat16, tag="h2")
                for g in range(2):
                    for k in range(4):
                        ptt = ptp.tile([128, 128], dt.float32, tag="pt")
                        nc.tensor.transpose(
                            ptt[:], s3[:, k, g * 128:(g + 1) * 128], eysf[:])
                        nc.scalar.activation(h2[:, g, k * 128:(k + 1) * 128],
                                             ptt[:], AF.Relu,
                                             bias=bns[:, 8 + g:9 + g],
                                             scale=bns[:, 6 + g:7 + g])
                pf = psp.tile([CLS, NPT], dt.float32, tag="pc")
                for g in range(2):
                    nc.tensor.matmul(pf[:], ows[:, g * CLS:(g + 1) * CLS],
                                     h2[:, g, :], start=(g == 0), stop=(g == 1))
                osb = wk.tile([CLS, NPT], dt.float32, tag="osb")
                nc.vector.tensor_scalar_add(osb[:], pf[:], bns[:CLS, 10:11])
                nc.sync.dma_start(out[:, tch * NPT:(tch + 1) * NPT], osb[:])

    nc.compile()
    return nc


# ----------------------------------------------------------------------------
# entry point
# ----------------------------------------------------------------------------

def kernel(**inputs):
    from concourse.bass_utils import run_bass_kernel_spmd

    if "nc" not in _CACHE:
        _CACHE["nc"] = _build_nc()
    nc = _CACHE["nc"]

    sh = _prep_shared(inputs)
    in_maps = []
    for c in range(NCORES):
        m = dict(sh)
        m.update(_prep_core(inputs, c))
        in_maps.append(m)

    res = run_bass_kernel_spmd(nc, in_maps, core_ids=list(range(NCORES)))
    outs = res.results
    full = np.zeros((B, CLS, N), np.float32)
    for c in range(NCORES):
        b, q = c // GRP, c % GRP
        full[b, :, q * QN:(q + 1) * QN] = outs[c]["out"]
    return full



# revision 7
# speedup vs baseline: 1.5705x; 1.5705x over previous
"""Trainium2 Bass kernel for nn_Decoder_17076789969159 (gnn_message_passing).

Sharding: data-parallel over batch (2 groups of 4 cores); within a group the
permuted point axis of each space-filling-curve order is split in 4 contiguous
chunks. Per-order conv outputs are AllGather'd inside the group; the next
layer gathers its inputs with host-composed inverse-permutation indices.

v2: all row gathers use the batched transposing SWDGE dma_gather (channel-
major results directly, one instruction per tile); conv outputs transpose
back to point-major with an SBUF-source dma_gather; stage 0 is built from a
constant interpolation-matrix matmul (no gathers).

Self-contained: hardcodes all shapes from the problem spec.
"""

import os
import numpy as np
import ml_dtypes

BF16 = ml_dtypes.bfloat16

# Problem shapes (hardcoded per contract)
B, N, LL, O, KK, PAD = 2, 32768, 8192, 3, 9, 4
C = 256          # conv output channels
C1R = 304        # conv1 input channels (256 xi + 48 low)
CH1 = 384        # padded h row (3 * 128)
CLS = 13
NCORES, GRP = 8, 4
QN = N // GRP            # 8192 permuted positions per core per order
NPT = 512                # points per conv tile
NT = QN // NPT           # 16 conv tiles per order per core
NTS0 = N // NPT          # 64 stage0 chunks (full batch, replicated in group)
NTF = QN // NPT          # 16 final tiles (core's original-index quarter)
NG = NPT + 2 * PAD       # 520 needed gather cols
NGI = 640                # gathered cols per conv tile (next mult of 128)
GW = NGI // 16           # idx cols per gather (40)
FW = NPT // 16           # idx cols per final gather (32)
NR = 34                  # xt rows per 128-pt interp group
EPS = 1e-5

_CACHE = {}


# ----------------------------------------------------------------------------
# host-side preparation
# ----------------------------------------------------------------------------

def _wrap16(vals):
    """index vector (len % 16 == 0) -> [128, len//16] int16 token layout."""
    v = np.asarray(vals, np.int16)
    w = v.reshape(-1, 16).T          # [16, len//16], slot j at [j%16, j//16]
    return np.tile(w, (8, 1))        # replicate across the 8 Q7 cores


def _bn_affine(g, b, m, v):
    s = g / np.sqrt(v + EPS)
    return s.astype(np.float32), (b - m * s).astype(np.float32)


def _interp_groups():
    """Per 128-pt group: xt row offset + [NR, 128] bf16 interp matrix."""
    pos = np.arange(N, dtype=np.float64) * ((LL - 1) / (N - 1))
    i0 = np.floor(pos).astype(np.int64)
    i1 = np.minimum(i0 + 1, LL - 1)
    t = np.round((pos - i0) * 256.0) / 256.0   # exact in bf16, as is 1-t
    r0s, mats = [], []
    for g in range(N // 128):
        sl = slice(g * 128, (g + 1) * 128)
        r0 = int(i0[sl].min())
        r0 = min(r0, LL - NR)
        span = int(max(i0[sl].max(), i1[sl].max())) - r0
        assert span < NR, (g, span)
        m = np.zeros((NR, 128), np.float32)
        for p in range(128):
            n = g * 128 + p
            m[i0[n] - r0, p] += 1.0 - t[n]
            m[i1[n] - r0, p] += t[n]
        r0s.append(r0)
        mats.append(m)
    return r0s, np.concatenate(mats, axis=1).astype(BF16)  # [NR, 256*128]


_IR0, _IMAT = _interp_groups()


def _prep_shared(inp):
    sh = {}
    w1 = np.asarray(inp["w1_w"], np.float32)   # [256, 304, 9]
    w1p = np.zeros((128, KK * 3 * C), np.float32)
    for j in range(KK):
        for kc in range(3):
            ci0 = kc * 128
            ncid = min(128, C1R - ci0)
            if ncid > 0:
                blk = w1[:, ci0:ci0 + ncid, j].T  # [ncid, 256]
                w1p[:ncid, (j * 3 + kc) * C:(j * 3 + kc) * C + C] = blk
    sh["w1p"] = w1p.astype(BF16)

    w2 = np.asarray(inp["w2_w"], np.float32)   # [256, 256, 9]
    w2p = np.zeros((128, KK * 2 * C), np.float32)
    for j in range(KK):
        for kc in range(2):
            blk = w2[:, kc * 128:(kc + 1) * 128, j].T
            w2p[:, (j * 2 + kc) * C:(j * 2 + kc) * C + C] = blk
    sh["w2p"] = w2p.astype(BF16)

    # conv1 with bn1 scale folded into the weights; bias via ones-row matmul
    s1, b1 = _bn_affine(inp["bn1_g"], inp["bn1_b"], inp["bn1_m"], inp["bn1_v"])
    c1 = np.asarray(inp["conv1_w"], np.float32) * s1[:, None]   # [48, 128]
    sh["c1"] = np.ascontiguousarray(c1.T).astype(BF16)          # [128, 48]
    b_hi = b1.astype(BF16).astype(np.float32)
    b_lo = (b1 - b_hi).astype(BF16)
    mrow = np.zeros((2, 128 + 48), np.float32)
    mrow[:, :128] = 1.0
    mrow[0, 128:] = b_hi
    mrow[1, 128:] = b_lo
    sh["mrow"] = mrow.astype(BF16)

    ow = np.asarray(inp["out_w"], np.float32)  # [13, 256]
    owp = np.zeros((128, 2 * CLS), np.float32)
    for g in range(2):
        owp[:, g * CLS:(g + 1) * CLS] = ow[:, g * 128:(g + 1) * 128].T
    sh["outw"] = owp.astype(BF16)

    bnv = np.zeros((128, 11), np.float32)
    sc, bc = _bn_affine(inp["bnc1_g"], inp["bnc1_b"], inp["bnc1_m"], inp["bnc1_v"])
    bc = bc + np.asarray(inp["w1_b"], np.float32) * sc
    for g in range(2):
        bnv[:, 2 + g] = sc[g * 128:(g + 1) * 128] / 3.0
        bnv[:, 4 + g] = bc[g * 128:(g + 1) * 128]
    sc2, bc2 = _bn_affine(inp["bnc2_g"], inp["bnc2_b"], inp["bnc2_m"], inp["bnc2_v"])
    bc2 = bc2 + np.asarray(inp["w2_b"], np.float32) * sc2
    for g in range(2):
        bnv[:, 6 + g] = sc2[g * 128:(g + 1) * 128] / 3.0
        bnv[:, 8 + g] = bc2[g * 128:(g + 1) * 128]
    bnv[:CLS, 10] = np.asarray(inp["out_b"], np.float32)
    sh["bnvec"] = bnv

    sh["smat"] = _IMAT                                    # [NR, 256*128]
    sh["ti"] = _wrap16(np.arange(256, dtype=np.int16))    # sbuf transpose idx
    return sh


def _prep_core(inp, c):
    b, q = c // GRP, c % GRP
    pc = {}
    x = np.asarray(inp["x"], np.float32)
    pc["xt"] = np.ascontiguousarray(x[b].T).astype(BF16)          # [8192, 256]
    pc["llf"] = np.asarray(inp["low_level_feat"], np.float32)[b].astype(BF16)

    rot = np.asarray(inp["rotations"], np.int64)[:, b, :]          # [O, N]
    inv = np.stack([np.argsort(rot[o], kind="stable") for o in range(O)])

    # geometry weights in permuted space, OOB taps zeroed
    coords = np.asarray(inp["coords"], np.float32)[b]              # [3, N]
    dist = np.asarray(inp["distances"], np.float32)[b]             # [O, N]
    wall = np.zeros((O, KK, N), np.float32)
    ar = np.arange(N)
    for o in range(O):
        co = coords[:, rot[o]]                                     # [3, N]
        d = dist[o]
        dp = np.pad(d, (PAD, PAD))
        cp = np.pad(co, ((0, 0), (PAD, PAD)))
        for j in range(KK):
            dd = (dp[j:j + N] - d) ** 2
            dc = ((cp[:, j:j + N] - co) ** 2).sum(0)
            w = np.exp(-(dd + dc))
            pin = ar + j - PAD
            w[(pin < 0) | (pin >= N)] = 0.0
            wall[o, j] = w
    # per-core tile-blocked, pre-broadcast to 128 partitions
    wgt = np.zeros((O * NT, KK * NPT), np.float32)
    for o in range(O):
        for tch in range(NT):
            base = q * QN + tch * NPT
            wgt[o * NT + tch] = wall[o, :, base:base + NPT].reshape(KK * NPT)
    pc["wgt"] = np.ascontiguousarray(
        np.broadcast_to(wgt.astype(BF16)[:, None, :], (O * NT, 128, KK * NPT)))

    # gather index tables (int16 token layout)
    mar = np.arange(NGI)
    l1cols, l2cols = [], []
    for o in range(O):
        for tch in range(NT):
            base = q * QN + tch * NPT
            pp = np.clip(base - PAD + mar, 0, N - 1)
            l1cols.append(_wrap16(rot[o][pp]))
            for os_ in range(O):
                l2cols.append(_wrap16(inv[os_][rot[o][pp]]))
    pc["l1i"] = np.concatenate(l1cols, axis=1)    # [128, O*NT*GW]
    pc["l2i"] = np.concatenate(l2cols, axis=1)    # [128, O*NT*O*GW]

    fcols = []
    for tch in range(NTF):
        base = q * QN + tch * NPT
        for os_ in range(O):
            fcols.append(_wrap16(inv[os_][base:base + NPT]))
    pc["fi"] = np.concatenate(fcols, axis=1)      # [128, NTF*O*FW]
    return pc


# ----------------------------------------------------------------------------
# device program
# ----------------------------------------------------------------------------

def _build_nc():
    import concourse.bacc as bacc
    import concourse.bass as bass
    import concourse.tile as tile
    import concourse.mybir as mybir

    dt = mybir.dt
    AF = mybir.ActivationFunctionType
    nocc = os.environ.get("KNOCC", "0") == "1"
    nc = bacc.Bacc("TRN2", target_bir_lowering=False, debug=False,
                   num_devices=1 if nocc else NCORES)

    def EIN(name, shape, dty):
        return nc.dram_tensor(name, list(shape), dty, kind="ExternalInput")

    xt = EIN("xt", [LL, C], dt.bfloat16)
    llf = EIN("llf", [128, N], dt.bfloat16)
    w1p = EIN("w1p", [128, KK * 3 * C], dt.bfloat16)
    w2p = EIN("w2p", [128, KK * 2 * C], dt.bfloat16)
    c1 = EIN("c1", [128, 48], dt.bfloat16)
    mrow = EIN("mrow", [2, 128 + 48], dt.bfloat16)
    outw = EIN("outw", [128, 2 * CLS], dt.bfloat16)
    bnvec = EIN("bnvec", [128, 11], dt.float32)
    smat = EIN("smat", [NR, 256 * 128], dt.bfloat16)
    wgt = EIN("wgt", [O * NT, 128, KK * NPT], dt.bfloat16)
    l1i = EIN("l1i", [128, O * NT * GW], dt.int16)
    l2i = EIN("l2i", [128, O * NT * O * GW], dt.int16)
    fi = EIN("fi", [128, NTF * O * FW], dt.int16)
    ti = EIN("ti", [128, 16], dt.int16)

    out = nc.dram_tensor("out", [CLS, QN], dt.float32, kind="ExternalOutput")
    dbg = os.environ.get("KDBG", "0") == "1"
    if dbg:
        dbg_h = nc.dram_tensor("dbg_h", [N, CH1], dt.bfloat16, kind="ExternalOutput")
        dbg_y1in0 = nc.dram_tensor("dbg_y1in0", [QN, C], dt.bfloat16, kind="ExternalOutput")
        dbg_y1all0 = nc.dram_tensor("dbg_y1all0", [N, C], dt.bfloat16, kind="ExternalOutput")
        dbg_y2in0 = nc.dram_tensor("dbg_y2in0", [QN, C], dt.bfloat16, kind="ExternalOutput")

    RG = [[0, 1, 2, 3], [4, 5, 6, 7]]

    with tile.TileContext(nc) as tc:
        with (
            tc.tile_pool(name="dram", bufs=1, space="DRAM") as dpool,
            tc.tile_pool(name="res", bufs=1) as res,
            tc.tile_pool(name="wk", bufs=3) as wk,
            tc.tile_pool(name="wc", bufs=2) as wc,
            tc.tile_pool(name="ps", bufs=6, space="PSUM") as psp,
        ):
            h_t = dpool.tile([N, CH1], dt.bfloat16, tag="h")
            y1in = [dpool.tile([QN, C], dt.bfloat16, tag=f"y1in{o}",
                               name=f"y1in{o}") for o in range(O)]
            y1all = [dpool.tile([N, C], dt.bfloat16, tag=f"y1all{o}",
                                name=f"y1all{o}") for o in range(O)]
            y2in = [dpool.tile([QN, C], dt.bfloat16, tag=f"y2in{o}",
                               name=f"y2in{o}") for o in range(O)]
            y2all = [dpool.tile([N, C], dt.bfloat16, tag=f"y2all{o}",
                                name=f"y2all{o}") for o in range(O)]

            def LOAD(src, shape, dty, tag):
                tl = res.tile(shape, dty, tag=tag, name=tag)
                nc.sync.dma_start(tl[:], src[:])
                return tl
            w1s = LOAD(w1p, [128, KK * 3 * C], dt.bfloat16, "w1s")
            w2s = LOAD(w2p, [128, KK * 2 * C], dt.bfloat16, "w2s")
            c1s = LOAD(c1, [128, 48], dt.bfloat16, "c1s")
            mrs = LOAD(mrow, [2, 128 + 48], dt.bfloat16, "mrs")
            ows = LOAD(outw, [128, 2 * CLS], dt.bfloat16, "ows")
            bns = LOAD(bnvec, [128, 11], dt.float32, "bns")
            l1s = LOAD(l1i, [128, O * NT * GW], dt.int16, "l1s")
            l2s = LOAD(l2i, [128, O * NT * O * GW], dt.int16, "l2s")
            fis = LOAD(fi, [128, NTF * O * FW], dt.int16, "fis")
            tis = LOAD(ti, [128, 16], dt.int16, "tis")

            def rows_pm(dram_tile, base, nrows, rowlen):
                """point-major SBUF tile [128, nrows//128, rowlen] <-> dram rows."""
                return bass.AP(dram_tile.tensor, base * rowlen,
                               [[rowlen, 128], [128 * rowlen, nrows // 128],
                                [1, rowlen]])

            def gatherT(dst, src_dram, idx_tile, col0, ncols, nidx, elem):
                nc.gpsimd.dma_gather(
                    out_ap=dst, in_ap=src_dram[:, :],
                    idxs_ap=idx_tile[:, col0:col0 + ncols],
                    num_idxs=nidx, num_idxs_reg=nidx, elem_size=elem,
                    transpose=True)

            def sbufT(dst, src):
                """[128, 2, 512] ch-major -> [128, 4, 256] point-major."""
                nc.gpsimd.dma_gather(
                    out_ap=dst, in_ap=src, idxs_ap=tis[:, :],
                    num_idxs=256, num_idxs_reg=256, elem_size=512,
                    transpose=True, sbuf_tokens_per_rank=128,
                    sbuf_free_dim_per_rank=1024)

            # ---------------- stage 0: build h ----------------
            for ch in range(NTS0):
                base = ch * NPT
                hrow = wk.tile([128, 4, CH1], dt.bfloat16, tag="hrow")
                smc = wk.tile([NR, 4, 128], dt.bfloat16, tag="smc")
                nc.sync.dma_start(smc[:], smat[:, ch * 512:(ch + 1) * 512])
                for g in range(4):
                    grp = ch * 4 + g
                    xr = wk.tile([NR, C], dt.bfloat16, tag="xr", name=f"xr{g}",
                                 bufs=8)
                    nc.sync.dma_start(xr[:], xt[_IR0[grp]:_IR0[grp] + NR, :])
                    ps = psp.tile([128, C], dt.float32, tag="pc", name="ps0")
                    nc.tensor.matmul(ps[:], smc[:, g, :],
                                     xr[:], start=True, stop=True)
                    nc.scalar.activation(hrow[:, g, 0:C], ps[:], AF.Copy)
                lsb = wk.tile([128, NPT], dt.bfloat16, tag="lsb")
                nc.sync.dma_start(lsb[:], llf[:, base:base + NPT])
                for g in range(4):
                    ps2 = psp.tile([128, 48], dt.float32, tag="pc", name="ps2")
                    nc.tensor.matmul(ps2[:], lsb[:, g * 128:(g + 1) * 128],
                                     c1s[:], start=True, stop=False)
                    nc.tensor.matmul(ps2[:], mrs[:, 0:128], mrs[:, 128:176],
                                     start=False, stop=True)
                    nc.scalar.activation(hrow[:, g, C:C + 48], ps2[:], AF.Relu)
                nc.vector.memset(hrow[:, :, C + 48:CH1], 0)
                nc.sync.dma_start(rows_pm(h_t, base, NPT, CH1), hrow[:])

            # ---------------- conv layer helper ----------------
            def conv_layer(yin, wsb_pack, nkc, gather_one, after_order=None):
                for o in range(O):
                    for tch in range(NT):
                        blk = o * NT + tch
                        hx = gather_one(o, tch)
                        wsb = wc.tile([128, KK * NPT], dt.bfloat16, tag="wsb")
                        nc.sync.dma_start(wsb[:], wgt[blk, :, :])
                        pg = [psp.tile([128, NPT], dt.float32, tag="pc",
                                       name=f"pg{g}") for g in range(2)]
                        for j in range(KK):
                            xw = wk.tile([128, nkc, NPT], dt.bfloat16, tag="xw")
                            for kc in range(nkc):
                                nc.vector.tensor_mul(
                                    xw[:, kc, :], hx[:, kc, j:j + NPT],
                                    wsb[:, j * NPT:(j + 1) * NPT])
                            for g in range(2):
                                for kc in range(nkc):
                                    wsl = wsb_pack[:, ((j * nkc + kc) * C + g * 128):
                                                   ((j * nkc + kc) * C + g * 128 + 128)]
                                    nc.tensor.matmul(
                                        pg[g][:], wsl, xw[:, kc, :],
                                        start=(j == 0 and kc == 0),
                                        stop=(j == KK - 1 and kc == nkc - 1))
                        ysb = wk.tile([128, 2, NPT], dt.bfloat16, tag="ysb")
                        for g in range(2):
                            nc.scalar.activation(ysb[:, g, :], pg[g][:], AF.Copy)
                        yT = wk.tile([128, 4, C], dt.bfloat16, tag="yT")
                        sbufT(yT[:], ysb[:])
                        nc.sync.dma_start(rows_pm(yin[o], tch * NPT, NPT, C), yT[:])
                    if after_order is not None:
                        after_order(o)

            # L1: transposing gather of padded-384 rows of h
            def gather_l1(o, tch):
                blk = o * NT + tch
                hx = wk.tile([128, 3, NGI], dt.bfloat16, tag="g1")
                gatherT(hx[:], h_t, l1s, blk * GW, GW, NGI, CH1)
                return hx

            def ag1(o):
                if nocc:
                    return
                nc.gpsimd.collective_compute(
                    "AllGather", mybir.AluOpType.bypass, replica_groups=RG,
                    ins=[y1in[o].opt()], outs=[y1all[o].opt()])

            conv_layer(y1in, w1s, 3, gather_l1, after_order=ag1)

            # L2: transposing gathers from 3 slabs, sum, bn1+relu
            def gather_l2(o, tch):
                blk = (o * NT + tch) * O
                gs = []
                for os_ in range(O):
                    g2 = wk.tile([128, 2, NGI], dt.bfloat16, tag="g2",
                                 name=f"g2o{os_}", bufs=6)
                    gatherT(g2[:], y1all[os_], l2s, (blk + os_) * GW, GW, NGI, C)
                    gs.append(g2)
                xs = wk.tile([128, 2, NGI], dt.bfloat16, tag="xs")
                nc.vector.tensor_add(xs[:], gs[0][:], gs[1][:])
                nc.vector.tensor_add(xs[:], xs[:], gs[2][:])
                hx = wk.tile([128, 2, NGI], dt.bfloat16, tag="hx2")
                for g in range(2):
                    nc.scalar.activation(hx[:, g, :], xs[:, g, :], AF.Relu,
                                         bias=bns[:, 4 + g:5 + g],
                                         scale=bns[:, 2 + g:3 + g])
                return hx

            def ag2(o):
                if nocc:
                    return
                nc.gpsimd.collective_compute(
                    "AllGather", mybir.AluOpType.bypass, replica_groups=RG,
                    ins=[y2in[o].opt()], outs=[y2all[o].opt()])

            conv_layer(y2in, w2s, 2, gather_l2, after_order=ag2)

            if dbg:
                nc.sync.dma_start(dbg_h[:, :], h_t[:, :])
                nc.sync.dma_start(dbg_y1in0[:, :], y1in[0][:, :])
                nc.sync.dma_start(dbg_y1all0[:, :], y1all[0][:, :])
                nc.sync.dma_start(dbg_y2in0[:, :], y2in[0][:, :])

            # ---------------- final: bn2+relu+proj ----------------
            for tch in range(NTF):
                gs = []
                for os_ in range(O):
                    blk = tch * O + os_
                    g3 = wk.tile([128, 2, NPT], dt.bfloat16, tag="g3",
                                 name=f"g3o{os_}", bufs=6)
                    gatherT(g3[:], y2all[os_], fis, blk * FW, FW, NPT, C)
                    gs.append(g3)
                xs = wk.tile([128, 2, NPT], dt.bfloat16, tag="xs3")
                nc.vector.tensor_add(xs[:], gs[0][:], gs[1][:])
                nc.vector.tensor_add(xs[:], xs[:], gs[2][:])
                h2 = wk.tile([128, 2, NPT], dt.bfloat16, tag="h2")
                for g in range(2):
                    nc.scalar.activation(h2[:, g, :], xs[:, g, :], AF.Relu,
                                         bias=bns[:, 8 + g:9 + g],
                                         scale=bns[:, 6 + g:7 + g])
                pf = psp.tile([CLS, NPT], dt.float32, tag="pc", name="pf")
                for g in range(2):
                    nc.tensor.matmul(pf[:], ows[:, g * CLS:(g + 1) * CLS],
                                     h2[:, g, :], start=(g == 0), stop=(g == 1))
                osb = wk.tile([CLS, NPT], dt.float32, tag="osb")
                nc.vector.tensor_scalar_add(osb[:], pf[:], bns[:CLS, 10:11])
                nc.sync.dma_start(out[:, tch * NPT:(tch + 1) * NPT], osb[:])

    nc.compile()
    return nc


# ----------------------------------------------------------------------------
# entry point
# ----------------------------------------------------------------------------

def kernel(**inputs):
    from concourse.bass_utils import run_bass_kernel_spmd

    if "nc" not in _CACHE:
        _CACHE["nc"] = _build_nc()
    nc = _CACHE["nc"]

    sh = _prep_shared(inputs)
    in_maps = []
    for c in range(NCORES):
        m = dict(sh)
        m.update(_prep_core(inputs, c))
        in_maps.append(m)

    res = run_bass_kernel_spmd(nc, in_maps, core_ids=list(range(NCORES)))
    outs = res.results
    full = np.zeros((B, CLS, N), np.float32)
    for c in range(NCORES):
        b, q = c // GRP, c % GRP
        full[b, :, q * QN:(q + 1) * QN] = outs[c]["out"]
    return full
